# revision 30
# baseline (speedup 1.0000x reference)
"""DRCLoss kernel for 8 Trainium2 NeuronCores (Bass/Tile, SPMD).

Math: loss = mean_i[ relu(l1_i + l2_i + d12_i - neg_i + 0.1) + max(l1_i, l2_i) ]
  where dist = cdist(ts, [ts; im1; im2]), l1/l2 are the block diagonals,
  neg is the min over non-self columns, d12 = ||im1 - im2 + 1e-6||.

Strategy (data parallel over rows, 512 rows/core):
  - One bf16 matmul per core computes nsq[i,j] = 2*x_i.r_j - ||x_i||^2 - ||r_j||^2
    = -||x_i - r_j||^2 directly in PSUM via 4 augmented contraction rows
    (row norms as bf16 high+low splits => norms are ~fp32 accurate).
  - tensor_mask_reduce performs the self-column exclusion and running max of
    nsq (== min of squared distance) in a single DVE op per PSUM tile, and
    also extracts the positive-pair diagonals.
  - Per-core column rotation (host side) puts each core's diagonal tiles at
    fixed tile indices, so a single SPMD program serves all 8 cores.
  - Host finishes with sqrt/relu/sum over 4096 rows in float64.
"""

import sys

if "/opt/trn_rl_repo" not in sys.path:
    sys.path.insert(0, "/opt/trn_rl_repo")

from contextlib import ExitStack

import ml_dtypes
import numpy as np

import concourse.bass as bass
import concourse.tile as tile
from concourse import mybir
from concourse.bass_utils import run_bass_kernel_spmd

BF16 = ml_dtypes.bfloat16
F32 = np.float32

B = 4096          # rows of feature_ts
D = 512           # feature dim
M = 8             # cores
BC = B // M       # rows per core (512)
NCOL = 3 * B      # columns of the distance matrix (12288)
K = D + 4         # contraction length incl. norm rows (516)
NT = NCOL // 512  # 512-wide column tiles per core (24)
NCH = 12          # DMA chunks (1024 cols each)
FLT_LOW = float(np.finfo(np.float32).min)

LAST_RESULTS = None  # BassKernelResults of the most recent run (for test.py)

_NC_CACHE = None


def _install_ntff_hook():
    """Provide antenv.axon_hooks (missing in this image) so trace=True can
    capture NTFF profiles through libaxon_pjrt.so."""
    try:
        import antenv.axon_hooks  # noqa: F401

        return
    except ImportError:
        pass
    try:
        import types

        import antenv
        from trn_agent_boot.trn_boot import _ntff_profile_via_ctypes

        mod = types.ModuleType("antenv.axon_hooks")
        mod._hook = None

        def set_axon_ntff_profile_hook(h):
            mod._hook = h

        def get_axon_ntff_profile_hook():
            return mod._hook

        mod.set_axon_ntff_profile_hook = set_axon_ntff_profile_hook
        mod.get_axon_ntff_profile_hook = get_axon_ntff_profile_hook
        sys.modules["antenv.axon_hooks"] = mod
        antenv.axon_hooks = mod
        hook = _ntff_profile_via_ctypes("/opt/axon/libaxon_pjrt.so")
        if hook is not None:
            mod._hook = hook
    except Exception:
        pass


def _split_multi_waits(nc):
    """This walrus build allows only ONE embedded sync wait per instruction.
    Hoist extra waits onto standalone EventSemaphore instructions inserted
    just before the owner (same engine, so program order is preserved)."""
    import bass_rust

    ctr = 0
    for blk in nc.m.functions[0].blocks:
        il = blk.instructions
        new = []
        for inst in il:
            si = getattr(inst, "sync_info", None)
            waits = list(si.on_wait) if si is not None else []
            if len(waits) > 1:
                for w in waits[:-1]:
                    ev = bass_rust.InstEventSemaphore(name=f"wsplit_{ctr}")
                    ctr += 1
                    ev.engine = inst.engine
                    ev.sync_info = bass_rust.SyncInfo(on_wait=[w], on_update=[])
                    new.append(ev)
                inst.sync_info = bass_rust.SyncInfo(
                    on_wait=[waits[-1]], on_update=list(si.on_update)
                )
            new.append(inst)
        il[:] = new


def _build_nc():
    """Build the SPMD Bass program (identical for all cores)."""
    nc = bass.Bass()
    f32 = mybir.dt.float32
    bf16 = mybir.dt.bfloat16

    rt_d = nc.dram_tensor("rt", [K, NCOL], bf16, kind="ExternalInput")
    lt_d = nc.dram_tensor("lt", [K, BC], bf16, kind="ExternalInput")
    imm_d = nc.dram_tensor("imm", [BC, 2 * D], f32, kind="ExternalInput")
    # cmask: [eye128 | negmask_0 | .. | negmask_3] along free dim
    cmask_d = nc.dram_tensor("cmask", [128, 128 + 4 * 512], f32, kind="ExternalInput")
    osq_d = nc.dram_tensor("osq", [128, 16], f32, kind="ExternalOutput")

    mx = mybir.AluOpType.max

    with ExitStack() as ctx:
        tc = ctx.enter_context(tile.TileContext(nc))
        const = ctx.enter_context(tc.tile_pool(name="const", bufs=1))
        scrp = ctx.enter_context(tc.tile_pool(name="scr", bufs=8))
        d12p = ctx.enter_context(tc.tile_pool(name="d12", bufs=4))
        psump = ctx.enter_context(tc.tile_pool(name="psum", bufs=7, space="PSUM"))
        warmp = ctx.enter_context(tc.tile_pool(name="warm", bufs=1, space="PSUM"))

        # NOTE: this walrus build only supports ONE embedded sync wait per
        # compute instruction. Every input DMA is therefore followed by a
        # tiny "absorber" op on its consumer engine (dummy matmul on PE,
        # 1-col copy on DVE) so each DMA semaphore is observed exactly once;
        # real instructions then carry at most one unobserved wait.
        pwarm = warmp.tile([128, 512], f32, tag="warm")

        osq = const.tile([128, 16], f32, tag="osq")

        # lhsT K-tiles: 4x[128, 512] + [4, 512]. lt0 is DMA'd first so the
        # HAM warmup burst (below) can start as early as possible.
        lt_t = []
        for k in range(5):
            kp = 128 if k < 4 else 4
            t = const.tile([kp, BC], bf16, tag=f"lt{k}", name=f"lt{k}")
            lt_t.append(t)

        def dma_lt(k):
            kp = 128 if k < 4 else 4
            nc.sync.dma_start(out=lt_t[k], in_=lt_d[k * 128 : k * 128 + kp, :])
            nc.tensor.matmul(
                pwarm[0:4, 0:8], lt_t[k][:, 0:4], lt_t[k][:, 0:8],
                start=True, stop=True,
            )

        dma_lt(0)
        # Sustained warmup burst on already-loaded lhsT data: ~5us of dense
        # matmuls releases the PE HAM clock throttle (4/8 -> 8/8) before the
        # first rhs chunk arrives, so real matmuls run at 2.4 GHz from the
        # start instead of warming up ~14us into the stream.
        for _ in range(16):
            nc.tensor.matmul(
                pwarm, lt_t[0][:, 0:128], lt_t[0], start=True, stop=True
            )
        for k in range(1, 5):
            dma_lt(k)

        # rhs K-tiles, chunked along columns so matmuls can start early
        CW = NCOL // NCH
        rt_t = {}

        def dma_chunk(ch):
            for k in range(5):
                kp = 128 if k < 4 else 4
                t = const.tile([kp, CW], bf16, tag=f"rt{k}_{ch}", name=f"rt{k}_{ch}")
                nc.sync.dma_start(
                    out=t,
                    in_=rt_d[k * 128 : k * 128 + kp, ch * CW : (ch + 1) * CW],
                )
                nc.tensor.matmul(
                    pwarm[0:4, 0:8], t[:, 0:4], t[:, 0:8], start=True, stop=True
                )
                rt_t[(k, ch)] = t

        dma_chunk(0)
        dma_chunk(1)

        cmask = const.tile([128, 128 + 4 * 512], f32, tag="cmask")
        nc.sync.dma_start(out=cmask, in_=cmask_d[:, :])
        cabs = const.tile([128, 128 + 4 * 512], f32, tag="cabs")
        nc.vector.tensor_copy(cabs, cmask)  # absorber: full subtile coverage
        eye = cmask[:, 0:128]
        negmask = [cmask[:, 128 + m * 512 : 128 + (m + 1) * 512] for m in range(4)]

        # [im1 | im2] natural row tiles for d12 (single DMA per row tile)
        imm_t = []
        for m in range(4):
            a = const.tile([128, 2 * D], f32, tag=f"imm_{m}", name=f"imm_{m}")
            nc.sync.dma_start(out=a, in_=imm_d[m * 128 : (m + 1) * 128, :])
            imm_t.append(a)

        for ch in range(2, NCH):
            dma_chunk(ch)

        colmax = [
            const.tile([128, 24], f32, tag=f"colmax{m}", name=f"colmax{m}")
            for m in range(4)
        ]

        # d12sq = sum((im1 + 1e-6 - im2)^2): independent of the matmul stream,
        # emitted early so its DVE ops carry few sync waits and overlap PE.
        for m in range(4):
            t1 = d12p.tile([128, D], f32, tag="d12a")
            nc.vector.tensor_sub(t1, imm_t[m][:, 0:D], imm_t[m][:, D : 2 * D])
            nc.vector.tensor_scalar_add(t1, t1, 1e-6)
            t2 = d12p.tile([128, D], f32, tag="d12b")
            nc.vector.tensor_mul(t2, t1, t1)
            nc.vector.tensor_reduce(
                out=osq[:, 4 * m + 3 : 4 * m + 4],
                in_=t2,
                axis=mybir.AxisListType.X,
                op=mybir.AluOpType.add,
            )

        for ch in range(NCH):
            for t_i in range(CW // 512):
                nt = ch * (CW // 512) + t_i
                for m in range(4):
                    psum = psump.tile([128, 512], f32, tag="ps")
                    for k in range(5):
                        nc.tensor.matmul(
                            psum,
                            lt_t[k][:, m * 128 : (m + 1) * 128],
                            rt_t[(k, ch)][:, t_i * 512 : (t_i + 1) * 512],
                            start=(k == 0),
                            stop=(k == 4),
                        )
                    if nt in (8, 16):
                        # positive-pair diagonal: l_sq = -nsq[p, m*128+p]
                        blk = nt // 8
                        scre = scrp.tile([128, 128], f32, tag="scre")
                        nc.vector.tensor_mul(
                            scre, psum[:, m * 128 : (m + 1) * 128], eye
                        )
                        nc.vector.tensor_reduce(
                            out=osq[:, 4 * m + blk : 4 * m + blk + 1],
                            in_=scre,
                            axis=mybir.AxisListType.X,
                            op=mybir.AluOpType.add,
                            negate=True,
                        )
                    if nt in (0, 8, 16):
                        # knock the self column out of the max (in place; an
                        # SBUF scratch slot's reuse would add a 2nd sync wait)
                        nc.vector.tensor_add(psum, psum, negmask[m])
                    nc.vector.tensor_reduce(
                        out=colmax[m][:, nt : nt + 1],
                        in_=psum,
                        axis=mybir.AxisListType.X,
                        op=mx,
                    )

        for m in range(4):
            # negsq = -max over the 24 per-tile maxima
            nc.vector.tensor_reduce(
                out=osq[:, 4 * m : 4 * m + 1],
                in_=colmax[m][:, 0:24],
                axis=mybir.AxisListType.X,
                op=mx,
                negate=True,
            )

        # SWDGE: a fresh queue, so the only wait is the DVE data dep
        # (HWDGE would add a queue-reuse wait and trip the 1-wait limit)
        nc.gpsimd.dma_start(out=osq_d[:, :], in_=osq)

    _split_multi_waits(nc)
    return nc


def _host_inputs(feature_ts, feature_image1, feature_image2):
    """Build the per-core input maps (bf16 splits, augmentation, rotation)."""
    ts = np.ascontiguousarray(feature_ts, dtype=np.float32)
    im1 = np.ascontiguousarray(feature_image1, dtype=np.float32)
    im2 = np.ascontiguousarray(feature_image2, dtype=np.float32)

    R = np.concatenate([ts, im1, im2], 0)  # [3B, D]
    rsq = (R.astype(np.float64) ** 2).sum(1)
    nrsqh = (-rsq).astype(BF16)
    nrsql = (-rsq - nrsqh.astype(np.float64)).astype(BF16)

    x_bf = ts.astype(BF16)
    r2_bf = (2.0 * R.astype(BF16).astype(np.float32)).astype(BF16)

    rt_full = np.empty((K, NCOL), dtype=BF16)
    rt_full[:D] = r2_bf.T
    rt_full[D] = BF16(1)
    rt_full[D + 1] = BF16(1)
    rt_full[D + 2] = nrsqh
    rt_full[D + 3] = nrsql

    # cmask: [eye128 | negmask_0..3]; negmask_m has -3e38 at (p, m*128+p)
    cmask = np.zeros((128, 128 + 4 * 512), dtype=np.float32)
    p = np.arange(128)
    cmask[p, p] = 1.0
    for m in range(4):
        cmask[p, 128 + m * 512 + m * 128 + p] = np.float32(-3.0e38)

    base = np.arange(B)
    in_maps = []
    for c in range(M):
        rows = slice(c * BC, (c + 1) * BC)
        perm = np.concatenate([b * B + (base + c * BC) % B for b in range(3)])
        lt = np.empty((K, BC), dtype=BF16)
        lt[:D] = x_bf[rows].T
        lt[D] = nrsqh[c * BC : (c + 1) * BC]
        lt[D + 1] = nrsql[c * BC : (c + 1) * BC]
        lt[D + 2] = BF16(1)
        lt[D + 3] = BF16(1)
        in_maps.append(
            {
                "rt": np.ascontiguousarray(rt_full[:, perm]),
                "lt": lt,
                "imm": np.concatenate([im1[rows], im2[rows]], axis=1),
                "cmask": cmask,
            }
        )
    return in_maps


def _combine(osq_list):
    """Host epilogue: sqrt/relu/sum in float64 over all 4096 rows."""
    trip_sum = 0.0
    for osq in osq_list:
        o = np.asarray(osq, dtype=np.float64)  # [128, 16]
        for m in range(4):
            negsq = o[:, 4 * m + 0]
            l1sq = o[:, 4 * m + 1]
            l2sq = o[:, 4 * m + 2]
            d12sq = o[:, 4 * m + 3]
            l1 = np.sqrt(np.maximum(l1sq, 0.0))
            l2 = np.sqrt(np.maximum(l2sq, 0.0))
            neg = np.sqrt(np.maximum(negsq, 0.0))
            d12 = np.sqrt(np.maximum(d12sq, 0.0))
            trip = np.maximum(l1 + l2 + d12 - neg + 0.1, 0.0) + np.maximum(l1, l2)
            trip_sum += trip.sum()
    return np.float32(trip_sum / B)


def kernel(feature_ts, feature_image1, feature_image2, _trace=False):
    global _NC_CACHE, LAST_RESULTS
    if _NC_CACHE is None:
        _NC_CACHE = _build_nc()
    if _trace:
        _install_ntff_hook()
    in_maps = _host_inputs(feature_ts, feature_image1, feature_image2)
    res = run_bass_kernel_spmd(_NC_CACHE, in_maps, list(range(M)), trace=_trace)
    LAST_RESULTS = res
    return _combine([res.results[c]["osq"] for c in range(M)])


# revision 32
# speedup vs baseline: 1.1369x; 1.1369x over previous
"""DRCLoss kernel for 8 Trainium2 NeuronCores (Bass/Tile, SPMD).

Math: loss = mean_i[ relu(l1_i + l2_i + d12_i - neg_i + 0.1) + max(l1_i, l2_i) ]
  where dist = cdist(ts, [ts; im1; im2]), l1/l2 are the block diagonals,
  neg is the min over non-self columns, d12 = ||im1 - im2 + 1e-6||.

Strategy (data parallel over rows, 512 rows/core):
  - One bf16 matmul per core computes nsq[i,j] = 2*x_i.r_j - ||x_i||^2 - ||r_j||^2
    = -||x_i - r_j||^2 directly in PSUM via 4 augmented contraction rows
    (row norms as bf16 high+low splits => norms are ~fp32 accurate).
  - tensor_mask_reduce performs the self-column exclusion and running max of
    nsq (== min of squared distance) in a single DVE op per PSUM tile, and
    also extracts the positive-pair diagonals.
  - Per-core column rotation (host side) puts each core's diagonal tiles at
    fixed tile indices, so a single SPMD program serves all 8 cores.
  - Host finishes with sqrt/relu/sum over 4096 rows in float64.
"""

import sys

if "/opt/trn_rl_repo" not in sys.path:
    sys.path.insert(0, "/opt/trn_rl_repo")

from contextlib import ExitStack

import ml_dtypes
import numpy as np

import concourse.bass as bass
import concourse.tile as tile
from concourse import mybir
from concourse.bass_utils import run_bass_kernel_spmd

BF16 = ml_dtypes.bfloat16
F32 = np.float32

B = 4096          # rows of feature_ts
D = 512           # feature dim
M = 8             # cores
BC = B // M       # rows per core (512)
NCOL = 3 * B      # columns of the distance matrix (12288)
K = D + 4         # contraction length incl. norm rows (516)
NT = NCOL // 512  # 512-wide column tiles per core (24)
NCH = 12          # DMA chunks (1024 cols each)
FLT_LOW = float(np.finfo(np.float32).min)

LAST_RESULTS = None  # BassKernelResults of the most recent run (for test.py)

_NC_CACHE = None


def _install_ntff_hook():
    """Provide antenv.axon_hooks (missing in this image) so trace=True can
    capture NTFF profiles through libaxon_pjrt.so."""
    try:
        import antenv.axon_hooks  # noqa: F401

        return
    except ImportError:
        pass
    try:
        import types

        import antenv
        from trn_agent_boot.trn_boot import _ntff_profile_via_ctypes

        mod = types.ModuleType("antenv.axon_hooks")
        mod._hook = None

        def set_axon_ntff_profile_hook(h):
            mod._hook = h

        def get_axon_ntff_profile_hook():
            return mod._hook

        mod.set_axon_ntff_profile_hook = set_axon_ntff_profile_hook
        mod.get_axon_ntff_profile_hook = get_axon_ntff_profile_hook
        sys.modules["antenv.axon_hooks"] = mod
        antenv.axon_hooks = mod
        hook = _ntff_profile_via_ctypes("/opt/axon/libaxon_pjrt.so")
        if hook is not None:
            mod._hook = hook
    except Exception:
        pass


def _split_multi_waits(nc):
    """This walrus build allows only ONE embedded sync wait per instruction.
    Hoist extra waits onto standalone EventSemaphore instructions inserted
    just before the owner (same engine, so program order is preserved)."""
    import bass_rust

    ctr = 0
    for blk in nc.m.functions[0].blocks:
        il = blk.instructions
        new = []
        for inst in il:
            si = getattr(inst, "sync_info", None)
            waits = list(si.on_wait) if si is not None else []
            if len(waits) > 1:
                for w in waits[:-1]:
                    ev = bass_rust.InstEventSemaphore(name=f"wsplit_{ctr}")
                    ctr += 1
                    ev.engine = inst.engine
                    ev.sync_info = bass_rust.SyncInfo(on_wait=[w], on_update=[])
                    new.append(ev)
                inst.sync_info = bass_rust.SyncInfo(
                    on_wait=[waits[-1]], on_update=list(si.on_update)
                )
            new.append(inst)
        il[:] = new


def _build_nc():
    """Build the SPMD Bass program (identical for all cores)."""
    nc = bass.Bass()
    f32 = mybir.dt.float32
    bf16 = mybir.dt.bfloat16

    rt_d = nc.dram_tensor("rt", [K, NCOL], bf16, kind="ExternalInput")
    lt_d = nc.dram_tensor("lt", [K, BC], bf16, kind="ExternalInput")
    imm_d = nc.dram_tensor("imm", [BC, 2 * D], f32, kind="ExternalInput")
    # cmask: [eye128 | negmask_0 | .. | negmask_3] along free dim
    cmask_d = nc.dram_tensor("cmask", [128, 128 + 4 * 512], bf16, kind="ExternalInput")
    osq_d = nc.dram_tensor("osq", [128, 16], f32, kind="ExternalOutput")

    mx = mybir.AluOpType.max

    with ExitStack() as ctx:
        tc = ctx.enter_context(tile.TileContext(nc))
        const = ctx.enter_context(tc.tile_pool(name="const", bufs=1))
        scrp = ctx.enter_context(tc.tile_pool(name="scr", bufs=8))
        d12p = ctx.enter_context(tc.tile_pool(name="d12", bufs=4))
        psump = ctx.enter_context(tc.tile_pool(name="psum", bufs=7, space="PSUM"))
        warmp = ctx.enter_context(tc.tile_pool(name="warm", bufs=1, space="PSUM"))

        # NOTE: this walrus build only supports ONE embedded sync wait per
        # compute instruction. Every input DMA is therefore followed by a
        # tiny "absorber" op on its consumer engine (dummy matmul on PE,
        # 1-col copy on DVE) so each DMA semaphore is observed exactly once;
        # real instructions then carry at most one unobserved wait.
        pwarm = warmp.tile([128, 512], f32, tag="warm")

        osq = const.tile([128, 16], f32, tag="osq")

        # lhsT K-tiles: 4x[128, 512] + [4, 512]. lt0 is DMA'd first so the
        # HAM warmup burst (below) can start as early as possible.
        lt_t = []
        for k in range(5):
            kp = 128 if k < 4 else 4
            t = const.tile([kp, BC], bf16, tag=f"lt{k}", name=f"lt{k}")
            lt_t.append(t)

        def dma_lt(k):
            kp = 128 if k < 4 else 4
            nc.sync.dma_start(out=lt_t[k], in_=lt_d[k * 128 : k * 128 + kp, :])
            nc.tensor.matmul(
                pwarm[0:4, 0:8], lt_t[k][:, 0:4], lt_t[k][:, 0:8],
                start=True, stop=True,
            )

        dma_lt(0)
        # Sustained warmup burst on already-loaded lhsT data: ~5us of dense
        # matmuls releases the PE HAM clock throttle (4/8 -> 8/8) before the
        # first rhs chunk arrives, so real matmuls run at 2.4 GHz from the
        # start instead of warming up ~14us into the stream.
        for _ in range(16):
            nc.tensor.matmul(
                pwarm, lt_t[0][:, 0:128], lt_t[0], start=True, stop=True
            )
        for k in range(1, 5):
            dma_lt(k)

        # rhs K-tiles, chunked along columns so matmuls can start early
        CW = NCOL // NCH
        rt_t = {}

        def dma_chunk(ch):
            for k in range(5):
                kp = 128 if k < 4 else 4
                t = const.tile([kp, CW], bf16, tag=f"rt{k}_{ch}", name=f"rt{k}_{ch}")
                nc.sync.dma_start(
                    out=t,
                    in_=rt_d[k * 128 : k * 128 + kp, ch * CW : (ch + 1) * CW],
                )
                nc.tensor.matmul(
                    pwarm[0:4, 0:8], t[:, 0:4], t[:, 0:8], start=True, stop=True
                )
                rt_t[(k, ch)] = t

        # bf16 keeps this off the critical rhs DMA path (values are exact:
        # 1.0 and -3e38 are bf16-representable)
        cmask = const.tile([128, 128 + 4 * 512], bf16, tag="cmask")
        nc.sync.dma_start(out=cmask, in_=cmask_d[:, :])
        cabs = const.tile([128, 128 + 4 * 512], bf16, tag="cabs")
        nc.vector.tensor_copy(cabs, cmask)  # absorber: full subtile coverage
        eye = cmask[:, 0:128]
        negmask = [cmask[:, 128 + m * 512 : 128 + (m + 1) * 512] for m in range(4)]

        dma_chunk(0)
        dma_chunk(1)

        for ch in range(2, 6):
            dma_chunk(ch)

        # [im1 | im2] natural row tiles for d12, mid-stream so they neither
        # delay the first rhs chunks nor starve the late ones
        imm_t = []
        for m in range(4):
            a = const.tile([128, 2 * D], f32, tag=f"imm_{m}", name=f"imm_{m}")
            nc.sync.dma_start(out=a, in_=imm_d[m * 128 : (m + 1) * 128, :])
            imm_t.append(a)

        for ch in range(6, NCH):
            dma_chunk(ch)

        colmax = [
            const.tile([128, 24], f32, tag=f"colmax{m}", name=f"colmax{m}")
            for m in range(4)
        ]

        for ch in range(NCH):
            for t_i in range(CW // 512):
                nt = ch * (CW // 512) + t_i
                for m in range(4):
                    psum = psump.tile([128, 512], f32, tag="ps")
                    for k in range(5):
                        nc.tensor.matmul(
                            psum,
                            lt_t[k][:, m * 128 : (m + 1) * 128],
                            rt_t[(k, ch)][:, t_i * 512 : (t_i + 1) * 512],
                            start=(k == 0),
                            stop=(k == 4),
                        )
                    if nt in (8, 16):
                        # positive-pair diagonal: l_sq = -nsq[p, m*128+p]
                        blk = nt // 8
                        scre = scrp.tile([128, 128], f32, tag="scre")
                        nc.vector.tensor_mul(
                            scre, psum[:, m * 128 : (m + 1) * 128], eye
                        )
                        nc.vector.tensor_reduce(
                            out=osq[:, 4 * m + blk : 4 * m + blk + 1],
                            in_=scre,
                            axis=mybir.AxisListType.X,
                            op=mybir.AluOpType.add,
                            negate=True,
                        )
                    if nt in (0, 8, 16):
                        # knock the self column out of the max (in place; an
                        # SBUF scratch slot's reuse would add a 2nd sync wait)
                        nc.vector.tensor_add(psum, psum, negmask[m])
                    nc.vector.tensor_reduce(
                        out=colmax[m][:, nt : nt + 1],
                        in_=psum,
                        axis=mybir.AxisListType.X,
                        op=mx,
                    )
                    if ch == 9 and t_i == 0:
                        # d12sq = sum((im1 + 1e-6 - im2)^2), slotted into the
                        # DVE stream late so the imm DMAs are long since done
                        t1 = d12p.tile([128, D], f32, tag="d12a")
                        nc.vector.tensor_sub(
                            t1, imm_t[m][:, 0:D], imm_t[m][:, D : 2 * D]
                        )
                        nc.vector.tensor_scalar_add(t1, t1, 1e-6)
                        t2 = d12p.tile([128, D], f32, tag="d12b")
                        nc.vector.tensor_mul(t2, t1, t1)
                        nc.vector.tensor_reduce(
                            out=osq[:, 4 * m + 3 : 4 * m + 4],
                            in_=t2,
                            axis=mybir.AxisListType.X,
                            op=mybir.AluOpType.add,
                        )

        for m in range(4):
            # negsq = -max over the 24 per-tile maxima
            nc.vector.tensor_reduce(
                out=osq[:, 4 * m : 4 * m + 1],
                in_=colmax[m][:, 0:24],
                axis=mybir.AxisListType.X,
                op=mx,
                negate=True,
            )

        # SWDGE: a fresh queue, so the only wait is the DVE data dep
        # (HWDGE would add a queue-reuse wait and trip the 1-wait limit)
        nc.gpsimd.dma_start(out=osq_d[:, :], in_=osq)

    _split_multi_waits(nc)
    return nc


def _host_inputs(feature_ts, feature_image1, feature_image2):
    """Build the per-core input maps (bf16 splits, augmentation, rotation)."""
    ts = np.ascontiguousarray(feature_ts, dtype=np.float32)
    im1 = np.ascontiguousarray(feature_image1, dtype=np.float32)
    im2 = np.ascontiguousarray(feature_image2, dtype=np.float32)

    R = np.concatenate([ts, im1, im2], 0)  # [3B, D]
    rsq = (R.astype(np.float64) ** 2).sum(1)
    nrsqh = (-rsq).astype(BF16)
    nrsql = (-rsq - nrsqh.astype(np.float64)).astype(BF16)

    x_bf = ts.astype(BF16)
    r2_bf = (2.0 * R.astype(BF16).astype(np.float32)).astype(BF16)

    rt_full = np.empty((K, NCOL), dtype=BF16)
    rt_full[:D] = r2_bf.T
    rt_full[D] = BF16(1)
    rt_full[D + 1] = BF16(1)
    rt_full[D + 2] = nrsqh
    rt_full[D + 3] = nrsql

    # cmask: [eye128 | negmask_0..3]; negmask_m has -3e38 at (p, m*128+p)
    cmask = np.zeros((128, 128 + 4 * 512), dtype=BF16)
    p = np.arange(128)
    cmask[p, p] = BF16(1.0)
    for m in range(4):
        cmask[p, 128 + m * 512 + m * 128 + p] = BF16(-3.0e38)

    base = np.arange(B)
    in_maps = []
    for c in range(M):
        rows = slice(c * BC, (c + 1) * BC)
        perm = np.concatenate([b * B + (base + c * BC) % B for b in range(3)])
        lt = np.empty((K, BC), dtype=BF16)
        lt[:D] = x_bf[rows].T
        lt[D] = nrsqh[c * BC : (c + 1) * BC]
        lt[D + 1] = nrsql[c * BC : (c + 1) * BC]
        lt[D + 2] = BF16(1)
        lt[D + 3] = BF16(1)
        in_maps.append(
            {
                "rt": np.ascontiguousarray(rt_full[:, perm]),
                "lt": lt,
                "imm": np.concatenate([im1[rows], im2[rows]], axis=1),
                "cmask": cmask,
            }
        )
    return in_maps


def _combine(osq_list):
    """Host epilogue: sqrt/relu/sum in float64 over all 4096 rows."""
    trip_sum = 0.0
    for osq in osq_list:
        o = np.asarray(osq, dtype=np.float64)  # [128, 16]
        for m in range(4):
            negsq = o[:, 4 * m + 0]
            l1sq = o[:, 4 * m + 1]
            l2sq = o[:, 4 * m + 2]
            d12sq = o[:, 4 * m + 3]
            l1 = np.sqrt(np.maximum(l1sq, 0.0))
            l2 = np.sqrt(np.maximum(l2sq, 0.0))
            neg = np.sqrt(np.maximum(negsq, 0.0))
            d12 = np.sqrt(np.maximum(d12sq, 0.0))
            trip = np.maximum(l1 + l2 + d12 - neg + 0.1, 0.0) + np.maximum(l1, l2)
            trip_sum += trip.sum()
    return np.float32(trip_sum / B)


def kernel(feature_ts, feature_image1, feature_image2, _trace=False):
    global _NC_CACHE, LAST_RESULTS
    if _NC_CACHE is None:
        _NC_CACHE = _build_nc()
    if _trace:
        _install_ntff_hook()
    in_maps = _host_inputs(feature_ts, feature_image1, feature_image2)
    res = run_bass_kernel_spmd(_NC_CACHE, in_maps, list(range(M)), trace=_trace)
    LAST_RESULTS = res
    return _combine([res.results[c]["osq"] for c in range(M)])


# revision 34
# speedup vs baseline: 1.5261x; 1.3424x over previous
"""DRCLoss kernel for 8 Trainium2 NeuronCores (Bass/Tile, SPMD).

Math: loss = mean_i[ relu(l1_i + l2_i + d12_i - neg_i + 0.1) + max(l1_i, l2_i) ]
  where dist = cdist(ts, [ts; im1; im2]), l1/l2 are the block diagonals,
  neg is the min over non-self columns, d12 = ||im1 - im2 + 1e-6||.

Strategy (data parallel over rows, 512 rows/core):
  - One bf16 matmul per core computes nsq[i,j] = 2*x_i.r_j - ||x_i||^2 - ||r_j||^2
    = -||x_i - r_j||^2 directly in PSUM via 4 augmented contraction rows
    (row norms as bf16 high+low splits => norms are ~fp32 accurate).
  - tensor_mask_reduce performs the self-column exclusion and running max of
    nsq (== min of squared distance) in a single DVE op per PSUM tile, and
    also extracts the positive-pair diagonals.
  - Per-core column rotation (host side) puts each core's diagonal tiles at
    fixed tile indices, so a single SPMD program serves all 8 cores.
  - Host finishes with sqrt/relu/sum over 4096 rows in float64.
"""

import sys

if "/opt/trn_rl_repo" not in sys.path:
    sys.path.insert(0, "/opt/trn_rl_repo")

from contextlib import ExitStack

import ml_dtypes
import numpy as np

import concourse.bass as bass
import concourse.tile as tile
from concourse import mybir
from concourse.bass_utils import run_bass_kernel_spmd

BF16 = ml_dtypes.bfloat16
F8 = ml_dtypes.float8_e4m3
F32 = np.float32

B = 4096          # rows of feature_ts
D = 512           # feature dim
M = 8             # cores
BC = B // M       # rows per core (512)
NCOL = 3 * B      # columns of the distance matrix (12288)
K = D + 4         # contraction length incl. norm rows (516)
NT = NCOL // 512  # 512-wide column tiles per core (24)
NCH = 12          # DMA chunks (1024 cols each)
FLT_LOW = float(np.finfo(np.float32).min)

LAST_RESULTS = None  # BassKernelResults of the most recent run (for test.py)

_NC_CACHE = None


def _install_ntff_hook():
    """Provide antenv.axon_hooks (missing in this image) so trace=True can
    capture NTFF profiles through libaxon_pjrt.so."""
    try:
        import antenv.axon_hooks  # noqa: F401

        return
    except ImportError:
        pass
    try:
        import types

        import antenv
        from trn_agent_boot.trn_boot import _ntff_profile_via_ctypes

        mod = types.ModuleType("antenv.axon_hooks")
        mod._hook = None

        def set_axon_ntff_profile_hook(h):
            mod._hook = h

        def get_axon_ntff_profile_hook():
            return mod._hook

        mod.set_axon_ntff_profile_hook = set_axon_ntff_profile_hook
        mod.get_axon_ntff_profile_hook = get_axon_ntff_profile_hook
        sys.modules["antenv.axon_hooks"] = mod
        antenv.axon_hooks = mod
        hook = _ntff_profile_via_ctypes("/opt/axon/libaxon_pjrt.so")
        if hook is not None:
            mod._hook = hook
    except Exception:
        pass


def _split_multi_waits(nc):
    """This walrus build allows only ONE embedded sync wait per instruction.
    Hoist extra waits onto standalone EventSemaphore instructions inserted
    just before the owner (same engine, so program order is preserved)."""
    import bass_rust

    ctr = 0
    for blk in nc.m.functions[0].blocks:
        il = blk.instructions
        new = []
        for inst in il:
            si = getattr(inst, "sync_info", None)
            waits = list(si.on_wait) if si is not None else []
            if len(waits) > 1:
                for w in waits[:-1]:
                    ev = bass_rust.InstEventSemaphore(name=f"wsplit_{ctr}")
                    ctr += 1
                    ev.engine = inst.engine
                    ev.sync_info = bass_rust.SyncInfo(on_wait=[w], on_update=[])
                    new.append(ev)
                inst.sync_info = bass_rust.SyncInfo(
                    on_wait=[waits[-1]], on_update=list(si.on_update)
                )
            new.append(inst)
        il[:] = new


def _build_nc():
    """Build the SPMD Bass program (identical for all cores).

    Per 512-row out-tile [128, 512]: two fp8-e4m3 DoubleRow matmuls cover the
    512 data contraction rows (K=256 each), one bf16 matmul adds the 4 norm
    rows; PSUM then holds nsq = -||x_i - r_j||^2. ScalarE copies each (masked)
    PSUM tile to SBUF fp16; VectorE folds a running elementwise max per row
    tile; the final reduce negates back to the squared min distance.
    """
    nc = bass.Bass()
    f32 = mybir.dt.float32
    f16 = mybir.dt.float16
    bf16 = mybir.dt.bfloat16
    f8 = mybir.dt.float8e4
    CW = NCOL // NCH
    DR = mybir.MatmulPerfMode.DoubleRow

    rt8_d = nc.dram_tensor("rt8", [NCH, 128, 4, CW], f8, kind="ExternalInput")
    rt4_d = nc.dram_tensor("rt4", [4, NCOL], bf16, kind="ExternalInput")
    lt8_d = nc.dram_tensor("lt8", [128, 4, BC], f8, kind="ExternalInput")
    lt4_d = nc.dram_tensor("lt4", [4, BC], bf16, kind="ExternalInput")
    imm_d = nc.dram_tensor("imm", [BC, 2 * D], f32, kind="ExternalInput")
    # cmask: [eye128 | negmask_0 | .. | negmask_3] along free dim
    cmask_d = nc.dram_tensor("cmask", [128, 128 + 4 * 512], bf16, kind="ExternalInput")
    osq_d = nc.dram_tensor("osq", [128, 16], f32, kind="ExternalOutput")

    mx = mybir.AluOpType.max

    with ExitStack() as ctx:
        tc = ctx.enter_context(tile.TileContext(nc))
        const = ctx.enter_context(tc.tile_pool(name="const", bufs=1))
        scrp = ctx.enter_context(tc.tile_pool(name="scr", bufs=8))
        d12p = ctx.enter_context(tc.tile_pool(name="d12", bufs=4))
        hpp = ctx.enter_context(tc.tile_pool(name="hp", bufs=6))
        psump = ctx.enter_context(tc.tile_pool(name="psum", bufs=7, space="PSUM"))
        warmp = ctx.enter_context(tc.tile_pool(name="warm", bufs=1, space="PSUM"))

        # NOTE: this walrus build only supports ONE embedded sync wait per
        # instruction (extras are hoisted by _split_multi_waits). Input DMAs
        # are still followed by tiny "absorber" ops on their consumer engine
        # so each DMA semaphore is observed once and real instructions rarely
        # need a hoisted wait.
        pwarm = warmp.tile([128, 512], f32, tag="warm")

        osq = const.tile([128, 16], f32, tag="osq")

        lt8 = const.tile([128, 4, BC], f8, tag="lt8")
        nc.sync.dma_start(out=lt8, in_=lt8_d[:, :, :])
        nc.tensor.matmul(
            pwarm[0:4, 0:8], lt8[:, 0, 0:4], lt8[:, 0, 0:8], start=True, stop=True
        )
        # Sustained warmup burst on already-loaded lhsT data: ~5us of dense
        # matmuls releases the PE HAM clock throttle (4/8 -> 8/8) before the
        # first rhs chunk arrives.
        for _ in range(16):
            nc.tensor.matmul(
                pwarm, lt8[:, 0, 0:128], lt8[:, 0, :], start=True, stop=True
            )
        lt4 = const.tile([4, BC], bf16, tag="lt4")
        nc.sync.dma_start(out=lt4, in_=lt4_d[:, :])
        nc.tensor.matmul(
            pwarm[0:4, 0:8], lt4[:, 0:4], lt4[:, 0:8], start=True, stop=True
        )

        # bf16 keeps this off the critical rhs DMA path
        cmask = const.tile([128, 128 + 4 * 512], bf16, tag="cmask")
        nc.sync.dma_start(out=cmask, in_=cmask_d[:, :])
        cabs = const.tile([128, 128 + 4 * 512], bf16, tag="cabs")
        nc.vector.tensor_copy(cabs, cmask)  # absorber: full subtile coverage
        eye = cmask[:, 0:128]
        negmask = [cmask[:, 128 + m * 512 : 128 + (m + 1) * 512] for m in range(4)]

        # per-row-tile running max accumulators (fp16)
        acch = []
        for m in range(4):
            a = const.tile([128, 512], f16, tag=f"acch{m}", name=f"acch{m}")
            nc.vector.memset(a, -60000.0)
            acch.append(a)

        rt8_t, rt4_t = {}, {}

        def dma_chunk(ch):
            t8 = const.tile([128, 4, CW], f8, tag=f"rt8_{ch}", name=f"rt8_{ch}")
            nc.sync.dma_start(out=t8, in_=rt8_d[ch])
            nc.tensor.matmul(
                pwarm[0:4, 0:8], t8[:, 0, 0:4], t8[:, 0, 0:8], start=True, stop=True
            )
            t4 = const.tile([4, CW], bf16, tag=f"rt4_{ch}", name=f"rt4_{ch}")
            nc.sync.dma_start(out=t4, in_=rt4_d[:, ch * CW : (ch + 1) * CW])
            nc.tensor.matmul(
                pwarm[0:4, 0:8], t4[:, 0:4], t4[:, 0:8], start=True, stop=True
            )
            rt8_t[ch], rt4_t[ch] = t8, t4

        for ch in range(6):
            dma_chunk(ch)

        # [im1 | im2] natural row tiles for d12, mid-stream so they neither
        # delay the first rhs chunks nor starve the late ones
        imm_t = []
        for m in range(4):
            a = const.tile([128, 2 * D], f32, tag=f"imm_{m}", name=f"imm_{m}")
            nc.sync.dma_start(out=a, in_=imm_d[m * 128 : (m + 1) * 128, :])
            imm_t.append(a)

        for ch in range(6, NCH):
            dma_chunk(ch)

        for ch in range(NCH):
            for t_i in range(CW // 512):
                nt = ch * (CW // 512) + t_i
                for m in range(4):
                    psum = psump.tile([128, 512], f32, tag="ps")
                    nc.tensor.matmul(
                        psum,
                        lt8[:, 0:2, m * 128 : (m + 1) * 128],
                        rt8_t[ch][:, 0:2, t_i * 512 : (t_i + 1) * 512],
                        start=True,
                        stop=False,
                        perf_mode=DR,
                    )
                    nc.tensor.matmul(
                        psum,
                        lt8[:, 2:4, m * 128 : (m + 1) * 128],
                        rt8_t[ch][:, 2:4, t_i * 512 : (t_i + 1) * 512],
                        start=False,
                        stop=False,
                        perf_mode=DR,
                    )
                    nc.tensor.matmul(
                        psum,
                        lt4[:, m * 128 : (m + 1) * 128],
                        rt4_t[ch][:, t_i * 512 : (t_i + 1) * 512],
                        start=False,
                        stop=True,
                    )
                    if nt in (8, 16):
                        # positive-pair diagonal: l_sq = -nsq[p, m*128+p]
                        blk = nt // 8
                        scre = scrp.tile([128, 128], f32, tag="scre")
                        nc.vector.tensor_mul(
                            scre, psum[:, m * 128 : (m + 1) * 128], eye
                        )
                        nc.vector.tensor_reduce(
                            out=osq[:, 4 * m + blk : 4 * m + blk + 1],
                            in_=scre,
                            axis=mybir.AxisListType.X,
                            op=mybir.AluOpType.add,
                            negate=True,
                        )
                    if nt in (0, 8, 16):
                        # knock the self column out of the max (in place)
                        nc.vector.tensor_add(psum, psum, negmask[m])
                    # ScalarE moves the tile to fp16 SBUF; VectorE folds the max
                    hp = hpp.tile([128, 512], f16, tag="hp")
                    nc.scalar.copy(hp, psum)
                    nc.vector.tensor_max(acch[m], acch[m], hp)
                    if ch == 9 and t_i == 0:
                        # d12sq = sum((im1 + 1e-6 - im2)^2), slotted into the
                        # DVE stream late so the imm DMAs are long since done
                        t1 = d12p.tile([128, D], f32, tag="d12a")
                        nc.vector.tensor_sub(
                            t1, imm_t[m][:, 0:D], imm_t[m][:, D : 2 * D]
                        )
                        nc.vector.tensor_scalar_add(t1, t1, 1e-6)
                        t2 = d12p.tile([128, D], f32, tag="d12b")
                        nc.vector.tensor_mul(t2, t1, t1)
                        nc.vector.tensor_reduce(
                            out=osq[:, 4 * m + 3 : 4 * m + 4],
                            in_=t2,
                            axis=mybir.AxisListType.X,
                            op=mybir.AluOpType.add,
                        )

        for m in range(4):
            # negsq = -max over the running fp16 maxima
            nc.vector.tensor_reduce(
                out=osq[:, 4 * m : 4 * m + 1],
                in_=acch[m],
                axis=mybir.AxisListType.X,
                op=mx,
                negate=True,
            )

        # SWDGE: a fresh queue, so the only wait is the DVE data dep
        nc.gpsimd.dma_start(out=osq_d[:, :], in_=osq)

    _split_multi_waits(nc)
    return nc


def _host_inputs(feature_ts, feature_image1, feature_image2):
    """Build the per-core input maps (fp8/bf16 casts, augmentation, rotation)."""
    ts = np.ascontiguousarray(feature_ts, dtype=np.float32)
    im1 = np.ascontiguousarray(feature_image1, dtype=np.float32)
    im2 = np.ascontiguousarray(feature_image2, dtype=np.float32)
    CW = NCOL // NCH

    R = np.concatenate([ts, im1, im2], 0)  # [3B, D]
    rsq = (R.astype(np.float64) ** 2).sum(1)
    nrsqh = (-rsq).astype(BF16)
    nrsql = (-rsq - nrsqh.astype(np.float64)).astype(BF16)

    x8 = ts.astype(F8)                                      # [B, D]
    r2_8 = (2.0 * R.astype(F8).astype(np.float32)).astype(F8)  # exact doubling

    # rt8 full: [128, 4, NCOL] with [p, j, n] = 2*R[n, j*128+p] (fp8)
    rt8_full = np.ascontiguousarray(
        r2_8.T.reshape(4, 128, NCOL).transpose(1, 0, 2)
    )
    rt4_full = np.empty((4, NCOL), dtype=BF16)
    rt4_full[0] = BF16(1)
    rt4_full[1] = BF16(1)
    rt4_full[2] = nrsqh
    rt4_full[3] = nrsql

    # cmask: [eye128 | negmask_0..3]; negmask_m has -3e38 at (p, m*128+p)
    cmask = np.zeros((128, 128 + 4 * 512), dtype=BF16)
    p = np.arange(128)
    cmask[p, p] = BF16(1.0)
    for m in range(4):
        # -30000: far below any real nsq (>= -9000) yet finite in fp16
        cmask[p, 128 + m * 512 + m * 128 + p] = BF16(-30000.0)

    base = np.arange(B)
    in_maps = []
    for c in range(M):
        rows = slice(c * BC, (c + 1) * BC)
        perm = np.concatenate([b * B + (base + c * BC) % B for b in range(3)])
        rt8_c = rt8_full[:, :, perm]                        # [128, 4, NCOL]
        rt8_c = np.ascontiguousarray(
            rt8_c.reshape(128, 4, NCH, CW).transpose(2, 0, 1, 3)
        )                                                   # [NCH, 128, 4, CW]
        lt8 = np.ascontiguousarray(
            x8[rows].T.reshape(4, 128, BC).transpose(1, 0, 2)
        )                                                   # [128, 4, BC]
        lt4 = np.empty((4, BC), dtype=BF16)
        lt4[0] = nrsqh[c * BC : (c + 1) * BC]
        lt4[1] = nrsql[c * BC : (c + 1) * BC]
        lt4[2] = BF16(1)
        lt4[3] = BF16(1)
        in_maps.append(
            {
                "rt8": rt8_c,
                "rt4": np.ascontiguousarray(rt4_full[:, perm]),
                "lt8": lt8,
                "lt4": lt4,
                "imm": np.concatenate([im1[rows], im2[rows]], axis=1),
                "cmask": cmask,
            }
        )
    return in_maps


def _combine(osq_list):
    """Host epilogue: sqrt/relu/sum in float64 over all 4096 rows."""
    trip_sum = 0.0
    for osq in osq_list:
        o = np.asarray(osq, dtype=np.float64)  # [128, 16]
        for m in range(4):
            negsq = o[:, 4 * m + 0]
            l1sq = o[:, 4 * m + 1]
            l2sq = o[:, 4 * m + 2]
            d12sq = o[:, 4 * m + 3]
            l1 = np.sqrt(np.maximum(l1sq, 0.0))
            l2 = np.sqrt(np.maximum(l2sq, 0.0))
            neg = np.sqrt(np.maximum(negsq, 0.0))
            d12 = np.sqrt(np.maximum(d12sq, 0.0))
            trip = np.maximum(l1 + l2 + d12 - neg + 0.1, 0.0) + np.maximum(l1, l2)
            trip_sum += trip.sum()
    return np.float32(trip_sum / B)


def kernel(feature_ts, feature_image1, feature_image2, _trace=False):
    global _NC_CACHE, LAST_RESULTS
    if _NC_CACHE is None:
        _NC_CACHE = _build_nc()
    if _trace:
        _install_ntff_hook()
    in_maps = _host_inputs(feature_ts, feature_image1, feature_image2)
    res = run_bass_kernel_spmd(_NC_CACHE, in_maps, list(range(M)), trace=_trace)
    LAST_RESULTS = res
    return _combine([res.results[c]["osq"] for c in range(M)])


# revision 35
# speedup vs baseline: 1.5751x; 1.0321x over previous
"""DRCLoss kernel for 8 Trainium2 NeuronCores (Bass/Tile, SPMD).

Math: loss = mean_i[ relu(l1_i + l2_i + d12_i - neg_i + 0.1) + max(l1_i, l2_i) ]
  where dist = cdist(ts, [ts; im1; im2]), l1/l2 are the block diagonals,
  neg is the min over non-self columns, d12 = ||im1 - im2 + 1e-6||.

Strategy (data parallel over rows, 512 rows/core):
  - One bf16 matmul per core computes nsq[i,j] = 2*x_i.r_j - ||x_i||^2 - ||r_j||^2
    = -||x_i - r_j||^2 directly in PSUM via 4 augmented contraction rows
    (row norms as bf16 high+low splits => norms are ~fp32 accurate).
  - tensor_mask_reduce performs the self-column exclusion and running max of
    nsq (== min of squared distance) in a single DVE op per PSUM tile, and
    also extracts the positive-pair diagonals.
  - Per-core column rotation (host side) puts each core's diagonal tiles at
    fixed tile indices, so a single SPMD program serves all 8 cores.
  - Host finishes with sqrt/relu/sum over 4096 rows in float64.
"""

import sys

if "/opt/trn_rl_repo" not in sys.path:
    sys.path.insert(0, "/opt/trn_rl_repo")

from contextlib import ExitStack

import ml_dtypes
import numpy as np

import concourse.bass as bass
import concourse.tile as tile
from concourse import mybir
from concourse.bass_utils import run_bass_kernel_spmd

BF16 = ml_dtypes.bfloat16
F8 = ml_dtypes.float8_e4m3
F32 = np.float32

B = 4096          # rows of feature_ts
D = 512           # feature dim
M = 8             # cores
BC = B // M       # rows per core (512)
NCOL = 3 * B      # columns of the distance matrix (12288)
K = D + 4         # contraction length incl. norm rows (516)
NT = NCOL // 512  # 512-wide column tiles per core (24)
NCH = 12          # DMA chunks (1024 cols each)
FLT_LOW = float(np.finfo(np.float32).min)

LAST_RESULTS = None  # BassKernelResults of the most recent run (for test.py)

_NC_CACHE = None


def _install_ntff_hook():
    """Provide antenv.axon_hooks (missing in this image) so trace=True can
    capture NTFF profiles through libaxon_pjrt.so."""
    try:
        import antenv.axon_hooks  # noqa: F401

        return
    except ImportError:
        pass
    try:
        import types

        import antenv
        from trn_agent_boot.trn_boot import _ntff_profile_via_ctypes

        mod = types.ModuleType("antenv.axon_hooks")
        mod._hook = None

        def set_axon_ntff_profile_hook(h):
            mod._hook = h

        def get_axon_ntff_profile_hook():
            return mod._hook

        mod.set_axon_ntff_profile_hook = set_axon_ntff_profile_hook
        mod.get_axon_ntff_profile_hook = get_axon_ntff_profile_hook
        sys.modules["antenv.axon_hooks"] = mod
        antenv.axon_hooks = mod
        hook = _ntff_profile_via_ctypes("/opt/axon/libaxon_pjrt.so")
        if hook is not None:
            mod._hook = hook
    except Exception:
        pass


def _split_multi_waits(nc):
    """This walrus build allows only ONE embedded sync wait per instruction.
    Hoist extra waits onto standalone EventSemaphore instructions inserted
    just before the owner (same engine, so program order is preserved)."""
    import bass_rust

    ctr = 0
    for blk in nc.m.functions[0].blocks:
        il = blk.instructions
        new = []
        for inst in il:
            si = getattr(inst, "sync_info", None)
            waits = list(si.on_wait) if si is not None else []
            if len(waits) > 1:
                for w in waits[:-1]:
                    ev = bass_rust.InstEventSemaphore(name=f"wsplit_{ctr}")
                    ctr += 1
                    ev.engine = inst.engine
                    ev.sync_info = bass_rust.SyncInfo(on_wait=[w], on_update=[])
                    new.append(ev)
                inst.sync_info = bass_rust.SyncInfo(
                    on_wait=[waits[-1]], on_update=list(si.on_update)
                )
            new.append(inst)
        il[:] = new


def _build_nc():
    """Build the SPMD Bass program (identical for all cores).

    Per 512-row out-tile [128, 512]: two fp8-e4m3 DoubleRow matmuls cover the
    512 data contraction rows (K=256 each), one bf16 matmul adds the 4 norm
    rows; PSUM then holds nsq = -||x_i - r_j||^2. ScalarE copies each (masked)
    PSUM tile to SBUF fp16; VectorE folds a running elementwise max per row
    tile; the final reduce negates back to the squared min distance.
    """
    nc = bass.Bass()
    f32 = mybir.dt.float32
    f16 = mybir.dt.float16
    bf16 = mybir.dt.bfloat16
    f8 = mybir.dt.float8e4
    CW = NCOL // NCH
    DR = mybir.MatmulPerfMode.DoubleRow

    rt8_d = nc.dram_tensor("rt8", [NCH, 128, 4, CW], f8, kind="ExternalInput")
    rt4_d = nc.dram_tensor("rt4", [4, NCOL], bf16, kind="ExternalInput")
    lt8_d = nc.dram_tensor("lt8", [128, 4, BC], f8, kind="ExternalInput")
    lt4_d = nc.dram_tensor("lt4", [4, BC], bf16, kind="ExternalInput")
    imm_d = nc.dram_tensor("imm", [BC, 2 * D], f32, kind="ExternalInput")
    # cmask: [eye128 | negmask_0 | .. | negmask_3] along free dim
    cmask_d = nc.dram_tensor("cmask", [128, 128 + 4 * 512], bf16, kind="ExternalInput")
    osq_d = nc.dram_tensor("osq", [128, 16], f32, kind="ExternalOutput")

    mx = mybir.AluOpType.max

    with ExitStack() as ctx:
        tc = ctx.enter_context(tile.TileContext(nc))
        const = ctx.enter_context(tc.tile_pool(name="const", bufs=1))
        scrp = ctx.enter_context(tc.tile_pool(name="scr", bufs=8))
        d12p = ctx.enter_context(tc.tile_pool(name="d12", bufs=4))
        hpp = ctx.enter_context(tc.tile_pool(name="hp", bufs=12))
        psump = ctx.enter_context(tc.tile_pool(name="psum", bufs=7, space="PSUM"))
        warmp = ctx.enter_context(tc.tile_pool(name="warm", bufs=1, space="PSUM"))

        # NOTE: this walrus build only supports ONE embedded sync wait per
        # instruction (extras are hoisted by _split_multi_waits). Input DMAs
        # are still followed by tiny "absorber" ops on their consumer engine
        # so each DMA semaphore is observed once and real instructions rarely
        # need a hoisted wait.
        pwarm = warmp.tile([128, 512], f32, tag="warm")

        osq = const.tile([128, 16], f32, tag="osq")

        lt8 = const.tile([128, 4, BC], f8, tag="lt8")
        nc.sync.dma_start(out=lt8, in_=lt8_d[:, :, :])
        nc.tensor.matmul(
            pwarm[0:4, 0:8], lt8[:, 0, 0:4], lt8[:, 0, 0:8], start=True, stop=True
        )
        # Sustained warmup burst on already-loaded lhsT data: ~5us of dense
        # matmuls releases the PE HAM clock throttle (4/8 -> 8/8) before the
        # first rhs chunk arrives.
        for _ in range(16):
            nc.tensor.matmul(
                pwarm, lt8[:, 0, 0:128], lt8[:, 0, :], start=True, stop=True
            )
        lt4 = const.tile([4, BC], bf16, tag="lt4")
        nc.sync.dma_start(out=lt4, in_=lt4_d[:, :])
        nc.tensor.matmul(
            pwarm[0:4, 0:8], lt4[:, 0:4], lt4[:, 0:8], start=True, stop=True
        )

        # bf16 keeps this off the critical rhs DMA path
        cmask = const.tile([128, 128 + 4 * 512], bf16, tag="cmask")
        nc.sync.dma_start(out=cmask, in_=cmask_d[:, :])
        cabs = const.tile([128, 128 + 4 * 512], bf16, tag="cabs")
        nc.vector.tensor_copy(cabs, cmask)  # absorber: full subtile coverage
        eye = cmask[:, 0:128]
        negmask = [cmask[:, 128 + m * 512 : 128 + (m + 1) * 512] for m in range(4)]

        # per-row-tile running max accumulators (fp16)
        acch = []
        for m in range(4):
            a = const.tile([128, 512], f16, tag=f"acch{m}", name=f"acch{m}")
            nc.vector.memset(a, -60000.0)
            acch.append(a)

        rt8_t, rt4_t = {}, {}

        def dma_chunk(ch):
            t8 = const.tile([128, 4, CW], f8, tag=f"rt8_{ch}", name=f"rt8_{ch}")
            nc.sync.dma_start(out=t8, in_=rt8_d[ch])
            nc.tensor.matmul(
                pwarm[0:4, 0:8], t8[:, 0, 0:4], t8[:, 0, 0:8], start=True, stop=True
            )
            t4 = const.tile([4, CW], bf16, tag=f"rt4_{ch}", name=f"rt4_{ch}")
            nc.sync.dma_start(out=t4, in_=rt4_d[:, ch * CW : (ch + 1) * CW])
            nc.tensor.matmul(
                pwarm[0:4, 0:8], t4[:, 0:4], t4[:, 0:8], start=True, stop=True
            )
            rt8_t[ch], rt4_t[ch] = t8, t4

        for ch in range(6):
            dma_chunk(ch)

        # [im1 | im2] natural row tiles for d12, mid-stream so they neither
        # delay the first rhs chunks nor starve the late ones
        imm_t = []
        for m in range(4):
            a = const.tile([128, 2 * D], f32, tag=f"imm_{m}", name=f"imm_{m}")
            nc.sync.dma_start(out=a, in_=imm_d[m * 128 : (m + 1) * 128, :])
            imm_t.append(a)

        for ch in range(6, NCH):
            dma_chunk(ch)

        for ch in range(NCH):
            for t_i in range(CW // 512):
                nt = ch * (CW // 512) + t_i
                # tail (norm rows, bf16) first for all four row tiles, then
                # the fp8 DoubleRow data matmuls back-to-back: 2 weight-mode
                # transitions per 512-column group instead of 8
                psums = []
                for m in range(4):
                    psum = psump.tile([128, 512], f32, tag="ps", name="psum")
                    psums.append(psum)
                    nc.tensor.matmul(
                        psum,
                        lt4[:, m * 128 : (m + 1) * 128],
                        rt4_t[ch][:, t_i * 512 : (t_i + 1) * 512],
                        start=True,
                        stop=False,
                    )
                for m in range(4):
                    nc.tensor.matmul(
                        psums[m],
                        lt8[:, 0:2, m * 128 : (m + 1) * 128],
                        rt8_t[ch][:, 0:2, t_i * 512 : (t_i + 1) * 512],
                        start=False,
                        stop=False,
                        perf_mode=DR,
                    )
                    nc.tensor.matmul(
                        psums[m],
                        lt8[:, 2:4, m * 128 : (m + 1) * 128],
                        rt8_t[ch][:, 2:4, t_i * 512 : (t_i + 1) * 512],
                        start=False,
                        stop=True,
                        perf_mode=DR,
                    )
                for m in range(4):
                    psum = psums[m]
                    if nt in (8, 16):
                        # positive-pair diagonal: l_sq = -nsq[p, m*128+p]
                        blk = nt // 8
                        scre = scrp.tile([128, 128], f32, tag="scre")
                        nc.vector.tensor_mul(
                            scre, psum[:, m * 128 : (m + 1) * 128], eye
                        )
                        nc.vector.tensor_reduce(
                            out=osq[:, 4 * m + blk : 4 * m + blk + 1],
                            in_=scre,
                            axis=mybir.AxisListType.X,
                            op=mybir.AluOpType.add,
                            negate=True,
                        )
                    if nt in (0, 8, 16):
                        # knock the self column out of the max (in place)
                        nc.vector.tensor_add(psum, psum, negmask[m])
                    # ScalarE moves the tile to fp16 SBUF; VectorE folds the max
                    hp = hpp.tile([128, 512], f16, tag="hp")
                    nc.scalar.copy(hp, psum)
                    nc.vector.tensor_max(acch[m], acch[m], hp)
                    if ch == 9 and t_i == 0:
                        # d12sq = sum((im1 + 1e-6 - im2)^2), slotted into the
                        # DVE stream late so the imm DMAs are long since done
                        t1 = d12p.tile([128, D], f32, tag="d12a")
                        nc.vector.tensor_sub(
                            t1, imm_t[m][:, 0:D], imm_t[m][:, D : 2 * D]
                        )
                        nc.vector.tensor_scalar_add(t1, t1, 1e-6)
                        t2 = d12p.tile([128, D], f32, tag="d12b")
                        nc.vector.tensor_mul(t2, t1, t1)
                        nc.vector.tensor_reduce(
                            out=osq[:, 4 * m + 3 : 4 * m + 4],
                            in_=t2,
                            axis=mybir.AxisListType.X,
                            op=mybir.AluOpType.add,
                        )

        for m in range(4):
            # negsq = -max over the running fp16 maxima
            nc.vector.tensor_reduce(
                out=osq[:, 4 * m : 4 * m + 1],
                in_=acch[m],
                axis=mybir.AxisListType.X,
                op=mx,
                negate=True,
            )

        # SWDGE: a fresh queue, so the only wait is the DVE data dep
        nc.gpsimd.dma_start(out=osq_d[:, :], in_=osq)

    _split_multi_waits(nc)
    return nc


def _host_inputs(feature_ts, feature_image1, feature_image2):
    """Build the per-core input maps (fp8/bf16 casts, augmentation, rotation)."""
    ts = np.ascontiguousarray(feature_ts, dtype=np.float32)
    im1 = np.ascontiguousarray(feature_image1, dtype=np.float32)
    im2 = np.ascontiguousarray(feature_image2, dtype=np.float32)
    CW = NCOL // NCH

    R = np.concatenate([ts, im1, im2], 0)  # [3B, D]
    rsq = (R.astype(np.float64) ** 2).sum(1)
    nrsqh = (-rsq).astype(BF16)
    nrsql = (-rsq - nrsqh.astype(np.float64)).astype(BF16)

    x8 = ts.astype(F8)                                      # [B, D]
    r2_8 = (2.0 * R.astype(F8).astype(np.float32)).astype(F8)  # exact doubling

    # rt8 full: [128, 4, NCOL] with [p, j, n] = 2*R[n, j*128+p] (fp8)
    rt8_full = np.ascontiguousarray(
        r2_8.T.reshape(4, 128, NCOL).transpose(1, 0, 2)
    )
    rt4_full = np.empty((4, NCOL), dtype=BF16)
    rt4_full[0] = BF16(1)
    rt4_full[1] = BF16(1)
    rt4_full[2] = nrsqh
    rt4_full[3] = nrsql

    # cmask: [eye128 | negmask_0..3]; negmask_m has -3e38 at (p, m*128+p)
    cmask = np.zeros((128, 128 + 4 * 512), dtype=BF16)
    p = np.arange(128)
    cmask[p, p] = BF16(1.0)
    for m in range(4):
        # -30000: far below any real nsq (>= -9000) yet finite in fp16
        cmask[p, 128 + m * 512 + m * 128 + p] = BF16(-30000.0)

    base = np.arange(B)
    in_maps = []
    for c in range(M):
        rows = slice(c * BC, (c + 1) * BC)
        perm = np.concatenate([b * B + (base + c * BC) % B for b in range(3)])
        rt8_c = rt8_full[:, :, perm]                        # [128, 4, NCOL]
        rt8_c = np.ascontiguousarray(
            rt8_c.reshape(128, 4, NCH, CW).transpose(2, 0, 1, 3)
        )                                                   # [NCH, 128, 4, CW]
        lt8 = np.ascontiguousarray(
            x8[rows].T.reshape(4, 128, BC).transpose(1, 0, 2)
        )                                                   # [128, 4, BC]
        lt4 = np.empty((4, BC), dtype=BF16)
        lt4[0] = nrsqh[c * BC : (c + 1) * BC]
        lt4[1] = nrsql[c * BC : (c + 1) * BC]
        lt4[2] = BF16(1)
        lt4[3] = BF16(1)
        in_maps.append(
            {
                "rt8": rt8_c,
                "rt4": np.ascontiguousarray(rt4_full[:, perm]),
                "lt8": lt8,
                "lt4": lt4,
                "imm": np.concatenate([im1[rows], im2[rows]], axis=1),
                "cmask": cmask,
            }
        )
    return in_maps


def _combine(osq_list):
    """Host epilogue: sqrt/relu/sum in float64 over all 4096 rows."""
    trip_sum = 0.0
    for osq in osq_list:
        o = np.asarray(osq, dtype=np.float64)  # [128, 16]
        for m in range(4):
            negsq = o[:, 4 * m + 0]
            l1sq = o[:, 4 * m + 1]
            l2sq = o[:, 4 * m + 2]
            d12sq = o[:, 4 * m + 3]
            l1 = np.sqrt(np.maximum(l1sq, 0.0))
            l2 = np.sqrt(np.maximum(l2sq, 0.0))
            neg = np.sqrt(np.maximum(negsq, 0.0))
            d12 = np.sqrt(np.maximum(d12sq, 0.0))
            trip = np.maximum(l1 + l2 + d12 - neg + 0.1, 0.0) + np.maximum(l1, l2)
            trip_sum += trip.sum()
    return np.float32(trip_sum / B)


def kernel(feature_ts, feature_image1, feature_image2, _trace=False):
    global _NC_CACHE, LAST_RESULTS
    if _NC_CACHE is None:
        _NC_CACHE = _build_nc()
    if _trace:
        _install_ntff_hook()
    in_maps = _host_inputs(feature_ts, feature_image1, feature_image2)
    res = run_bass_kernel_spmd(_NC_CACHE, in_maps, list(range(M)), trace=_trace)
    LAST_RESULTS = res
    return _combine([res.results[c]["osq"] for c in range(M)])
